# revision 59
# baseline (speedup 1.0000x reference)
"""Discretized-mixture NLL loss kernel for Trainium2 (Bass/Tile), 8-core data parallel.

v4.1: midpoint-pdf formulation. The bin probability is approximated by the
midpoint rule (error ~(delta*s2)^2; validated 2.1e-3 max rel err vs 2e-2 gate):
    dcdf ~= 2*delta * dPhi/dv|_xe = COEF * s2 * exp(-A^2),  COEF = delta*2/sqrt(pi)
    A = (mu - xe) * s2 ;  s2 = exp(-8*tanh(ls/8))/sqrt(2);  g = exp(pi)
    num = sum_m COEF*(g*s2)*E + EPS*den ;  den = sum_m g ;  nll = ln(den)-ln(num)
vs v3 this removes the erf pair, hi/lo STTs, dlt, and PSUM->SBUF copies, and
uses only table sets {exp,tanh,square} + {ln} => 2 ACT table loads, no
mid-kernel table barriers.

DVE partition-alignment rules honored (BIR verifier: tensor_tensor inputs must
share partitions; scalar_tensor_tensor is exempt but always 1x):
  - psum layout D(0:30)|t-dest(32:64)|pi(64:94)|ls(96:126) so exp emits
    s2g=[s2(0:32)|g(32:64)] and A = ps[0:32]*s2g[0:32] is an aligned TT.
  - gs = g*s2 via STT (base-free) into gs_stack[32q] aligning with E[32q],
    making qd = gs*E an aligned bf16 2x TT written over s2 rows.
Per image (4 supertiles): stacked ACT sq=A^2, E=exp(-sq) at [128,1024].
Reduction: per supertile one dense MM pair (K=64, M=128): lhsT maps
num->rows 4j+c (COEF on qd rows 0:32, EPS on g rows 32:64) and den->64+4j+c;
all 16 supertiles accumulate into ONE psum [128,1024]. Tail: two Lns into
column-adjacent halves, one aligned subtract, one 256 KB out DMA (host drops
4j+3 pad rows).
"""

import numpy as np
from ml_dtypes import bfloat16

WIDTH = 512
C_IMG = 3
N_MIX = 10
SIZE = 64
STD = 127.5
EPS = 1e-8
DELTA = 1.0 / STD / 2.0
COEF = DELTA * 2.0 / np.sqrt(np.pi)
LOG_INV_SQRT2 = -0.34657359027997264
N_CORES = 8
SUP_W = 1024          # pixels per supertile
SUB_W = 512           # matmul moving-dim tile


def make_consts(W, b):
    """Host-side prep of the small constant tensors."""
    W = np.asarray(W, np.float32)
    b = np.asarray(b, np.float32)
    # main GEMM lhsT: psum rows D(0:30)|ls(32:62)|t-dest(64:96)|pi(96:126)
    # W rows: pi 0:30, mu 30:60, ls 60:90
    wrow = np.zeros((128, WIDTH), np.float32)
    wrow[0:30] = W[30:60]       # D rows get mu weights
    wrow[32:62] = W[60:90]      # ls
    wrow[96:126] = W[0:30]      # pi
    wt = np.zeros((4, 128, 128), np.float32)    # [k, kk, m]
    for k in range(4):
        wt[k] = wrow[:, 128 * k:128 * (k + 1)].T
    wt = np.ascontiguousarray(wt.transpose(1, 0, 2).reshape(128, 512)).astype(bfloat16)
    # bias lhsT, K=16 (x16 rows 4*img+c = xe_c of image img; row 15 = ones),
    # one [16, 128] variant per image: D rows get -xe_c + b_mu
    bx = np.zeros((4, 16, 128), np.float32)
    for i in range(4):
        for r in range(30):
            bx[i, 4 * i + r % 3, r] = -1.0
            bx[i, 15, r] = b[30 + r]
    bx = np.ascontiguousarray(bx.transpose(1, 0, 2).reshape(16, 512)).astype(bfloat16)
    # dense reduction lhsTs: per supertile j [64, 128]; rhs rows qd(0:32)|g(32:64)
    lred = np.zeros((16, 64, 128), np.float32)
    for j in range(16):
        for r in range(30):
            c = r % 3
            lred[j, r, 4 * j + c] = COEF             # qd rows -> num
            lred[j, 32 + r, 4 * j + c] = EPS         # g rows -> num eps*den part
            lred[j, 32 + r, 64 + 4 * j + c] = 1.0    # g rows -> den
            lred[j, 32 + r, 4 * j + 3] = 1.0         # pad cols: den-like, Ln finite
            lred[j, 32 + r, 64 + 4 * j + 3] = 1.0
    lred = np.ascontiguousarray(lred.transpose(1, 0, 2).reshape(64, 2048)).astype(bfloat16)
    # exp per-row (scale, bias): out rows 0:32 t->s2, rows 32:64 pi->g
    scb = np.zeros((64, 2), np.float32)
    scb[0:32, 0] = -8.0
    scb[0:32, 1] = LOG_INV_SQRT2
    scb[32:64, 0] = 1.0
    scb[32:62, 1] = b[0:30]
    # tanh per-row bias: b_ls / 8
    tb = np.zeros((32, 1), np.float32)
    tb[0:30, 0] = b[60:90] / 8.0
    return wt, bx, lred, scb, tb


def build_nc(n_batch=4):
    """Build the single-core Bass program (same NEFF runs SPMD on all cores)."""
    from contextlib import ExitStack

    import concourse.bacc as bacc
    import concourse.mybir as mybir
    import concourse.tile as tile

    f32 = mybir.dt.float32
    bf16 = mybir.dt.bfloat16
    fp8 = mybir.dt.float8e4
    ALU = mybir.AluOpType
    ACT = mybir.ActivationFunctionType

    assert n_batch == 4, "kernel hardcodes nb=4 (16 supertiles, 128 red rows)"
    PX_IMG = SIZE * SIZE                        # 4096
    S = n_batch * PX_IMG // SUP_W               # supertiles per core (16)

    nc = bacc.Bacc("TRN2", target_bir_lowering=False, debug=False)
    # pz is supertile-major: [img, q, kk, 1024*k + px] so ONE 1 MB transfer
    # (8 KB/partition) delivers all four K-chunks of one supertile
    pz = nc.dram_tensor("pz", [n_batch, 4, 128, PX_IMG], fp8, kind="ExternalInput").ap()
    x4 = nc.dram_tensor("x4", [16, PX_IMG], bf16, kind="ExternalInput").ap()
    wt = nc.dram_tensor("wt", [128, 512], bf16, kind="ExternalInput").ap()
    bx = nc.dram_tensor("bx", [16, 512], bf16, kind="ExternalInput").ap()
    lred = nc.dram_tensor("lred", [64, 2048], bf16, kind="ExternalInput").ap()
    scb = nc.dram_tensor("scb", [64, 2], f32, kind="ExternalInput").ap()
    tb = nc.dram_tensor("tb", [32, 1], f32, kind="ExternalInput").ap()
    out = nc.dram_tensor("out", [64, SUP_W], f32, kind="ExternalOutput").ap()

    with tile.TileContext(nc) as tc, ExitStack() as ctx:
        const_pool = ctx.enter_context(tc.tile_pool(name="const", bufs=1))
        xt_pool = ctx.enter_context(tc.tile_pool(name="xt", bufs=9))
        s2g_pool = ctx.enter_context(tc.tile_pool(name="s2g", bufs=5))
        cg_pool = ctx.enter_context(tc.tile_pool(name="cg", bufs=2))
        a_pool = ctx.enter_context(tc.tile_pool(name="ast", bufs=2))
        gs_pool = ctx.enter_context(tc.tile_pool(name="gst", bufs=2))
        sq_pool = ctx.enter_context(tc.tile_pool(name="sq", bufs=2))
        e_pool = ctx.enter_context(tc.tile_pool(name="e", bufs=2))
        ln_pool = ctx.enter_context(tc.tile_pool(name="ln", bufs=1))
        nll_pool = ctx.enter_context(tc.tile_pool(name="nll", bufs=1))
        ps_pool = ctx.enter_context(tc.tile_pool(name="ps", bufs=3, space="PSUM"))
        red_pool = ctx.enter_context(tc.tile_pool(name="red", bufs=1, space="PSUM"))

        red_t = red_pool.tile([128, SUP_W], f32, tag="red", name="red")

        def warmup_pe():
            # throwaway matmuls on a memset scratch (no DMA dependency!):
            # keeps the PE busy through the HAM activity window during the
            # DMA ramp so the first real GEMMs run at 2.4 GHz instead of 1.2.
            # A tiny exp also pre-triggers the exp_and_others table load.
            wu_t = const_pool.tile([128, SUB_W], bf16)
            nc.vector.memset(wu_t[:], 0.0)
            wl_t = const_pool.tile([32, 2], f32)
            nc.scalar.activation(wl_t[:], wu_t[0:32, 0:2], ACT.Exp)
            wp = ps_pool.tile([128, SUB_W], f32, tag="ps", name="warm")
            for r in range(12):
                nc.tensor.matmul(wp[0:128, :], wu_t[:, 0:128],
                                 wu_t[:, 0:512], start=True, stop=True)

        xts = {}

        def load_image(img):
            ts = [None] * 4
            for q in range(4):
                ts[q] = xt_pool.tile([128, PX_IMG], fp8, tag="xt",
                                     name=f"xt{img}_{q}")
                nc.sync.dma_start(ts[q][:], pz[img, q])
            xts[img] = ts

        # warmup first so its ops head every engine FIFO; the consts the first
        # supertile chain needs (wt for its GEMM, scb/tb for tanh/exp) go on
        # the sync ring just ahead of image 0; the slow 4-partition x4 plus
        # bx/lred ride the scalar ring in parallel
        warmup_pe()
        x4_sb = const_pool.tile([16, PX_IMG], bf16)
        nc.scalar.dma_start(x4_sb[:], x4)
        wt_sb = const_pool.tile([128, 512], bf16)
        nc.scalar.dma_start(wt_sb[:], wt)
        bx_sb = const_pool.tile([16, 512], bf16)
        nc.scalar.dma_start(bx_sb[:], bx)
        scb_sb = const_pool.tile([64, 2], f32)
        nc.scalar.dma_start(scb_sb[:], scb)
        tb_sb = const_pool.tile([32, 1], f32)
        nc.scalar.dma_start(tb_sb[:], tb)
        lred_sb = const_pool.tile([64, 2048], bf16)
        nc.scalar.dma_start(lred_sb[:], lred)
        load_image(0)

        def phase1(sup, a_t, gs_t, s2g_t):
            # s2g_t is a [64, 2048] PAIR tile shared by supertiles (q, q^1):
            # this supertile uses columns 1024*(q%2).
            img, q = divmod(sup, 4)
            xt = xts[img][q]
            cb = SUP_W * (q % 2)
            cs = slice(cb, cb + SUP_W)
            ps = ps_pool.tile([128, SUP_W], f32, tag="ps")
            for t in range(2):
                nc.tensor.matmul(
                    ps[:, SUB_W * t:SUB_W * (t + 1)],
                    bx_sb[:, 128 * img:128 * (img + 1)],
                    x4_sb[:, SUP_W * q + SUB_W * t:SUP_W * q + SUB_W * (t + 1)],
                    start=True, stop=False,
                )
            for k in range(4):
                for t in range(2):
                    nc.tensor.matmul(
                        ps[:, SUB_W * t:SUB_W * (t + 1)],
                        wt_sb[:, 128 * k:128 * (k + 1)],
                        xt[:, SUP_W * k + SUB_W * t:SUP_W * k + SUB_W * (t + 1)],
                        start=False, stop=(k == 3),
                    )
            # t = tanh(ls/8 + b_ls/8): rows 32:64 -> 64:96
            nc.scalar.activation(ps[64:96, :], ps[32:64, :], ACT.Tanh,
                                 scale=0.125, bias=tb_sb[:, 0:1])
            # s2g = exp(rowwise scale/bias on [t | pi]) -> [s2(0:32) | g(32:64)] bf16
            nc.scalar.activation(s2g_t[0:64, cs], ps[64:128, :], ACT.Exp,
                                 bias=scb_sb[:, 1:2], scale=scb_sb[:, 0:1])
            # A = D * s2 (aligned TT: both base 0), bf16 into per-image stack
            nc.vector.tensor_tensor(a_t[32 * q:32 * (q + 1), :], ps[0:32, :],
                                    s2g_t[0:32, cs], ALU.mult)

        def pair_gs(img, q0, gs_t, s2g_t):
            # one copy re-bases both sups' g rows to partitions 0:32, then
            # aligned bf16 TTs write gs into the stack rows 32q
            cg_t = cg_pool.tile([32, 2 * SUP_W], bf16, tag="cg")
            nc.vector.tensor_copy(cg_t[:], s2g_t[32:64, :])
            for q in (q0, q0 + 1):
                cs = slice(SUP_W * (q % 2), SUP_W * (q % 2 + 1))
                nc.vector.tensor_tensor(gs_t[32 * q:32 * (q + 1), :],
                                        cg_t[0:32, cs], s2g_t[0:32, cs],
                                        ALU.mult)

        def finish_sqe(img, a_t):
            # sq = A^2, E = exp(-sq) bf16
            sq_t = sq_pool.tile([128, SUP_W], f32, tag="sq")
            nc.scalar.activation(sq_t[:], a_t[:], ACT.Square)
            e_t = e_pool.tile([128, SUP_W], bf16, tag="e")
            nc.scalar.activation(e_t[:], sq_t[:], ACT.Exp, scale=-1.0)
            return e_t

        def finish_red(img, gs_t, e_t, s2gs):
            # qd = gs*E (aligned bf16 TT, overwrites that sup's s2 columns)
            # then dense reduction MMs
            for q in range(4):
                j = 4 * img + q
                s2g_t = s2gs[q // 2]
                cb = SUP_W * (q % 2)
                nc.vector.tensor_tensor(
                    s2g_t[0:32, cb:cb + SUP_W], gs_t[32 * q:32 * (q + 1), :],
                    e_t[32 * q:32 * (q + 1), :], ALU.mult)
                for t in range(2):
                    nc.tensor.matmul(
                        red_t[:, SUB_W * t:SUB_W * (t + 1)],
                        lred_sb[:, 128 * j:128 * (j + 1)],
                        s2g_t[0:64, cb + SUB_W * t:cb + SUB_W * (t + 1)],
                        start=(j == 0), stop=(j == S - 1),
                    )

        # software pipeline: finish(img-1) interleaves INSIDE phase1(img) so
        # each engine FIFO keeps img's work ahead of img-1's dependent ops
        # (sq/E after 2 supertiles' tanh+exp; qd/reds after all 4 GEMMs)
        ctxs = []
        es = {}
        for img in range(n_batch):
            if img + 1 < n_batch:
                load_image(img + 1)
            if img >= 1:
                # emit prev's sq/E here: ACT is DMA-gated at image starts, so
                # the wait is free, and the DVE sem bump lands right after
                # prev's last A-TT instead of after later qd work
                es[img - 1] = finish_sqe(*ctxs[img - 1][:2])
            a_t = a_pool.tile([128, SUP_W], f32, tag="ast", name=f"ast{img}")
            gs_t = gs_pool.tile([128, SUP_W], bf16, tag="gst", name=f"gst{img}")
            s2gs = [s2g_pool.tile([64, 2 * SUP_W], bf16, tag="s2g",
                                  name=f"s2g{img}_{p}")
                    for p in range(2)]
            for q in range(4):
                phase1(4 * img + q, a_t, gs_t, s2gs[q // 2])
                if q % 2 == 1:
                    pair_gs(img, q - 1, gs_t, s2gs[q // 2])
            if 1 <= img < n_batch - 1:
                finish_red(ctxs[img - 1][0], ctxs[img - 1][2], es[img - 1],
                           ctxs[img - 1][3])
            ctxs.append((img, a_t, gs_t, s2gs))
        # last image: sq/E first so its DVE sem bump isn't coalesced behind
        # the previous image's qd block, then both remaining red groups
        last = n_batch - 1
        es[last] = finish_sqe(*ctxs[last][:2])
        finish_red(ctxs[last - 1][0], ctxs[last - 1][2], es[last - 1],
                   ctxs[last - 1][3])
        finish_red(ctxs[last][0], ctxs[last][2], es[last], ctxs[last][3])

        # tail: Lns into column-adjacent halves, aligned subtract; split into
        # two column halves so nll/out-DMA of half 0 overlap Lns of half 1
        ln_t = ln_pool.tile([64, 2 * SUP_W], f32, tag="lnt")
        nll_t = nll_pool.tile([64, SUP_W], f32, tag="nll")
        for h in range(2):
            cs = slice(SUB_W * h, SUB_W * (h + 1))
            nc.scalar.activation(ln_t[:, SUB_W * h:SUB_W * (h + 1)],
                                 red_t[0:64, cs], ACT.Ln)
            nc.scalar.activation(ln_t[:, SUP_W + SUB_W * h:SUP_W + SUB_W * (h + 1)],
                                 red_t[64:128, cs], ACT.Ln)
            nc.vector.tensor_tensor(
                nll_t[:, cs], ln_t[:, SUP_W + SUB_W * h:SUP_W + SUB_W * (h + 1)],
                ln_t[:, SUB_W * h:SUB_W * (h + 1)], ALU.subtract)
            nc.sync.dma_start(out[:, cs], nll_t[:, cs])

    nc.compile()
    return nc


def prep_core_inputs(px_z_shard, x_shard, consts):
    """px_z_shard [nb,512,64,64] f32, x_shard [nb,64,64,3] f32 -> input map."""
    wt, bx, lred, scb, tb = consts
    nb = px_z_shard.shape[0]
    # supertile-major: [img, q, kk, 1024*k + px]
    from ml_dtypes import float8_e4m3
    arr = px_z_shard.reshape(nb, 4, 128, 4, SUP_W)     # img, k, kk, q, px
    pzs = np.ascontiguousarray(
        arr.transpose(0, 3, 2, 1, 4).reshape(nb, 4, 128, SIZE * SIZE)
    ).astype(float8_e4m3)
    # x16: rows 4*img+c = xe_c of image img, row 15 = ones; cols 1024*q+px
    xq = x_shard.reshape(nb, 4, SUP_W, C_IMG)
    x4 = np.zeros((16, SIZE * SIZE), np.float32)
    for i in range(nb):
        x4[4 * i:4 * i + 3] = xq[i].transpose(2, 0, 1).reshape(C_IMG, SIZE * SIZE)
    x4[15, :] = 1.0
    return {
        "pz": pzs, "x4": x4.astype(bfloat16), "wt": wt, "bx": bx,
        "lred": lred, "scb": scb, "tb": tb,
    }


def gather_core_output(o, nb):
    """o [64, 1024] f32 (row 4j+c with j=4*img+q, col px) -> [nb, 64, 64, 3]."""
    o4 = o.reshape(nb, 4, 4, SUP_W)[:, :, 0:3]      # img, q, c, px
    o4 = o4.transpose(0, 1, 3, 2)                    # img, q, px, c
    return np.ascontiguousarray(o4).reshape(nb, SIZE, SIZE, C_IMG)


_NC_CACHE = {}


def kernel(px_z, x, W, b):
    from concourse.bass_utils import run_bass_kernel_spmd

    px_z = np.asarray(px_z, np.float32)
    x = np.asarray(x, np.float32)
    B = px_z.shape[0]
    nb = B // N_CORES
    consts = make_consts(W, b)
    key = (nb,)
    if key not in _NC_CACHE:
        _NC_CACHE[key] = build_nc(n_batch=nb)
    nc = _NC_CACHE[key]
    in_maps = [
        prep_core_inputs(px_z[nb * i:nb * (i + 1)], x[nb * i:nb * (i + 1)], consts)
        for i in range(N_CORES)
    ]
    res = run_bass_kernel_spmd(nc, in_maps, core_ids=list(range(N_CORES)))
    outs = [gather_core_output(res.results[i]["out"], nb) for i in range(N_CORES)]
    return np.concatenate(outs, 0)
